# revision 19
# baseline (speedup 1.0000x reference)
"""Trainium2 Bass kernel for nn_NeuralHSMM (8-core SPMD, data-parallel over batch).

Per core: 2 sequences. States live on the 128 partitions throughout.
 - Small derived HSMM params (A, Dhat, emission GEMM operands, logpi)
   are computed host-side in float32 ("replicate the small params"):
   each core receives 1/8 of the packed [K, 824] f32 param block and
   an AllGather reassembles it on device — wire cost 52 KB/core
   instead of the 56 MB of raw ctx_* weights.
 - x ships as fp8-e4m3 (quarter of f32 wire), converted to f32 on
   device before the emission GEMMs (loglik rel-err ~4e-4).
 - emission log-probs via PE GEMMs; C = cumsum_t(log_b) per state.
 - forward scan: duration logsumexp = free-dim max/exp + fused
   multiply-reduce against rotated exp(logD); transition logsumexp =
   exp-domain PE matmul against A = exp(logA). Normalizer = bulk
   cumsum of per-step max_k log_b, re-anchored to the exact measured
   max every L_MEAS steps via PE transpose (only cross-partition op).
"""
import os
import sys
import numpy as np

sys.path.insert(0, "/opt/trn_rl_repo")

import jax

# Persistent compilation cache: the axon PJRT path re-lowers and
# re-compiles the wrapped NEFF custom-call on every invocation (fresh
# closure -> jit cache miss). The persistent cache turns that ~200ms
# recompile into a disk read.
try:
    jax.config.update("jax_compilation_cache_dir", "/tmp/jax_cc_cache")
    # Entry-size floor keeps small host-side (XLA:CPU) jits out of the
    # cache; only the ~113KB NEFF-wrapped device executable is cached.
    jax.config.update("jax_persistent_cache_min_entry_size_bytes", 50000)
    jax.config.update("jax_persistent_cache_min_compile_time_secs", 0.0)
except Exception:
    pass

from contextlib import ExitStack

import concourse.bass as bass
import concourse.bacc as bacc
import concourse.mybir as mybir
import concourse.tile as tile

K = 128
DMAX = 48
NF = 256
CDIM = 256
B = 16
T = 768
NCORES = 8
BL = B // NCORES          # 2 sequences per core
TT = BL * T               # 1536
NEG = -1.0e9
LOG2PI = float(np.log(2.0 * np.pi))
L_MEAS = 4
f32 = mybir.dt.float32
f8 = mybir.dt.float8e4
AX = mybir.AxisListType
ALU = mybir.AluOpType
ACTF = mybir.ActivationFunctionType

# packed f32 param layout: [K, PW]
O_A = 0                    # A = exp(logA)            [K, K]
O_D = O_A + K              # Dhat = exp(logD)         [K, DMAX]
O_M1 = O_D + DMAX          # M1T = blockwise inv^T    [K, NF]
O_M2 = O_M1 + NF           # M2T = blockwise (-2*mu_eff*inv)^T [K, NF]
O_I = O_M2 + NF            # identity                 [K, K]
O_S = O_I + K              # bias_k, lpT, mxlp, mxlp  [K, 4]
PW = O_S + 8               # pad to multiple of NCORES
CW = PW // NCORES          # 103

_CACHE = {}


def build_program(ncores=NCORES):
    nc = bacc.Bacc(
        "TRN2",
        target_bir_lowering=False,
        debug=False,
        num_devices=ncores,
    )

    def dp(name, shape, dt=f32, out=False):
        return nc.declare_dram_parameter(name, shape, dt, isOutput=out)

    XB = TT * NF                       # x bytes in the blob
    blob = dp("blob", [XB + 4 * K * CW], f8)
    x_l = blob[0:XB].rearrange("(t f) -> t f", f=NF)
    pk_flat = blob[XB:].bitcast(f32)   # [K * CW] f32
    out_m = dp("out_m", [K, BL + 1], out=True)

    cc_in = nc.dram_tensor("cc_in", [K * CW], f32)
    cc_out = nc.dram_tensor("cc_out", [ncores * K * CW], f32,
                            addr_space="Shared")
    groups = [list(range(ncores))]

    with tile.TileContext(nc) as tc, ExitStack() as ctx:
        per = ctx.enter_context(tc.tile_pool(name="per", bufs=1))
        tmp = ctx.enter_context(tc.tile_pool(name="tmp", bufs=2))
        pst = ctx.enter_context(tc.tile_pool(name="pst", bufs=2, space="PSUM"))

        dma = nc.sync.dma_start
        dmag = nc.gpsimd.dma_start

        # ---------- params (host-derived, packed, AllGathered) ----------
        dmag(cc_in[:], pk_flat)
        nc.gpsimd.collective_compute(
            "AllGather", ALU.bypass, replica_groups=groups,
            ins=[cc_in[:]], outs=[cc_out[:]])
        cc3 = cc_out[:].rearrange("(r k w) -> r k w", k=K, w=CW)
        pk = per.tile([K, PW], f32)
        for r in range(ncores):
            dma(pk[:, r * CW:(r + 1) * CW], cc3[r])

        A_sb = pk[:, O_A:O_A + K]
        Dhat = pk[:, O_D:O_D + DMAX]
        M1T = pk[:, O_M1:O_M1 + NF]
        M2T = pk[:, O_M2:O_M2 + NF]
        ident = pk[:, O_I:O_I + K]
        bias_k = pk[:, O_S:O_S + 1]
        lpT = pk[:, O_S + 1:O_S + 2]
        mxlp = per.tile([1, 2], f32)
        nc.vector.tensor_copy(mxlp[:], pk[0:1, O_S + 2:O_S + 4])

        # ---------- emissions ----------
        NT = TT // K
        with tc.tile_pool(name="em", bufs=3) as em, \
             tc.tile_pool(name="emp", bufs=2, space="PSUM") as emp:
            xT = [per.tile([K, TT], f32, name=f"xT{c}", tag=f"xT{c}") for c in range(2)]
            sqT = [per.tile([K, TT], f32, name=f"sqT{c}", tag=f"sqT{c}") for c in range(2)]
            for r in range(NT):
                xtb = em.tile([K, NF], f8, tag="xtb")
                dma(xtb[:], x_l[r * K:(r + 1) * K, :])
                xt = em.tile([K, NF], f32, tag="xt")
                nc.vector.tensor_copy(xt[:], xtb[:])
                for c in range(2):
                    pp = emp.tile([K, K], f32, tag="em")
                    nc.tensor.transpose(pp[:], xt[:, c * K:(c + 1) * K],
                                        ident)
                    nc.vector.tensor_copy(xT[c][:, r * K:(r + 1) * K], pp[:])
                    nc.scalar.activation(sqT[c][:, r * K:(r + 1) * K], pp[:],
                                         ACTF.Square)
            log_b = per.tile([K, TT], f32)
            for b_ in range(TT // 512):
                sl = slice(b_ * 512, (b_ + 1) * 512)
                acc = emp.tile([K, 512], f32, tag="em", name="acc")
                nc.tensor.matmul(acc[:], M1T[:, 0:K], sqT[0][:, sl],
                                 start=True, stop=False)
                nc.tensor.matmul(acc[:], M1T[:, K:NF], sqT[1][:, sl],
                                 start=False, stop=False)
                nc.tensor.matmul(acc[:], M2T[:, 0:K], xT[0][:, sl],
                                 start=False, stop=False)
                nc.tensor.matmul(acc[:], M2T[:, K:NF], xT[1][:, sl],
                                 start=False, stop=True)
                nc.scalar.activation(log_b[:, sl], acc[:], ACTF.Identity,
                                     scale=-0.5, bias=bias_k)

            dcols = per.tile([K, NT], f32)
            for r in range(NT):
                pp = emp.tile([K, K], f32, tag="em")
                nc.tensor.transpose(pp[:], log_b[:, r * K:(r + 1) * K],
                                    ident)
                nc.vector.tensor_reduce(dcols[:, r:r + 1], pp[:], axis=AX.X,
                                        op=ALU.max)
            dP = emp.tile([NT, K], f32, tag="em", name="dP")
            nc.tensor.transpose(dP[:], dcols[:], ident)
            dT = per.tile([NT, K], f32)
            nc.vector.tensor_copy(dT[:], dP[:])

        d2 = per.tile([BL, T], f32)
        for s in range(BL):
            for b_ in range(T // K):
                dma(d2[s:s + 1, b_ * K:(b_ + 1) * K],
                    dT[s * (T // K) + b_:s * (T // K) + b_ + 1, :])
        cum2 = per.tile([BL, T], f32)
        zb2 = per.tile([BL, T], f32)
        nc.vector.memset(zb2[:], 0.0)
        nc.vector.tensor_tensor_scan(cum2[:], d2[:], zb2[:], 0.0,
                                     op0=ALU.add, op1=ALU.add)

        C_il = per.tile([K, TT], f32)
        zbT = per.tile([K, T], f32)
        nc.vector.memset(zbT[:], 0.0)
        for s in range(BL):
            nc.vector.tensor_tensor_scan(
                C_il[:, s::2], log_b[:, s * T:(s + 1) * T], zbT[:], 0.0,
                op0=ALU.add, op1=ALU.add)
        cum2b = per.tile([1, T], f32)
        dma(cum2b[:], cum2[1:2, :])
        cumb = per.tile([K, TT], f32)
        nc.gpsimd.partition_broadcast(cumb[:, 0::2], cum2[0:1, :])
        nc.gpsimd.partition_broadcast(cumb[:, 1::2], cum2b[:])
        CC = per.tile([K, TT], f32)
        nc.vector.tensor_sub(CC[:], C_il[:], cumb[:])

        # ---------- Dhat rotations (pair-duplicated, reversed windows) ----------
        REV2 = per.tile([K, 192], f32)
        for i in range(96):
            csrc = (47 - i) % 48
            nc.vector.tensor_copy(
                REV2[:, 2 * i:2 * i + 2],
                Dhat[:, csrc:csrc + 1].broadcast_to((K, 2)))
        Drot = per.tile([K, 48 * 96], f32)
        for c in range(48):
            nc.vector.tensor_copy(Drot[:, c * 96:(c + 1) * 96],
                                  REV2[:, 2 * c:2 * c + 96])

        # ---------- scan state init ----------
        rb = per.tile([K, 2 * DMAX], f32)
        nc.vector.memset(rb[:], NEG)
        nc.vector.tensor_copy(rb[:, 2 * DMAX - 2:2 * DMAX],
                              lpT.broadcast_to((K, 2)))
        SM = per.tile([K, 2], f32)
        P = per.tile([K, 2], f32)
        OUTC = per.tile([BL, 1], f32)
        ZERO2 = per.tile([2, K], f32)
        nc.vector.memset(ZERO2[:], 0.0)
        BT = [per.tile([2, 1], f32, name=f"BT{j}", tag=f"BT{j}") for j in range(2)]
        TB = [per.tile([K, 2], f32, name=f"TB{j}", tag=f"TB{j}") for j in range(2)]
        nc.gpsimd.partition_broadcast(BT[0][:], mxlp[:, 0:1])
        nc.gpsimd.partition_broadcast(TB[0][:], mxlp[:])

        # ---------- scan ----------
        loop = ctx.enter_context(tc.tile_pool(name="loop", bufs=4))
        qpool = ctx.enter_context(tc.tile_pool(name="qp", bufs=4, space="PSUM"))
        tpool = ctx.enter_context(tc.tile_pool(name="tp", bufs=2, space="PSUM"))

        def step(CCW, c2w, u):
            cur = (u // L_MEAS) % 2
            c = (48 - u) % 48
            MXN = loop.tile([K, 2], f32, tag="MXN")
            nc.vector.tensor_reduce(
                MXN[:], rb[:].rearrange("p (j s) -> p s j", s=2),
                axis=AX.X, op=ALU.max, negate=True)
            X = loop.tile([K, 98], f32, tag="X")
            for s in range(2):
                nc.vector.tensor_scalar_add(
                    X[:, s:96:2], rb[:, s::2], MXN[:, s:s + 1])
            U1 = loop.tile([K, 2], f32, tag="U1")
            nc.vector.tensor_sub(U1[:], CCW[:, 2 * u:2 * u + 2], TB[cur][:])
            nc.vector.tensor_sub(X[:, 96:98], U1[:], MXN[:])
            E = loop.tile([K, 98], f32, tag="E")
            nc.scalar.activation(E[:], X[:], ACTF.Exp)
            scr = loop.tile([K, 96], f32, tag="scr")
            for s in range(2):
                nc.vector.scalar_tensor_tensor(
                    scr[:, s::2], E[:, s:96:2], 1.0,
                    Drot[:, c * 96 + s:(c + 1) * 96:2],
                    op0=ALU.mult, op1=ALU.mult,
                    accum_out=SM[:, s:s + 1])
            nc.vector.tensor_mul(P[:], SM[:], E[:, 96:98])
            QP = qpool.tile([K, 2], f32, tag="QP")
            nc.tensor.matmul(QP[:], A_sb, P[:], start=True, stop=True)
            LQ = loop.tile([K, 2], f32, tag="LQ")
            nc.scalar.activation(LQ[:], QP[:], ACTF.Ln)
            nc.vector.tensor_sub(rb[:, 2 * u:2 * u + 2], LQ[:], U1[:])
            if u == DMAX - 1:
                nc.vector.tensor_add(OUTC[:], BT[cur][:], c2w[:, u:u + 1])
            if u % L_MEAS == L_MEAS - 1:
                nxt = 1 - cur
                SP1 = tpool.tile([2, K], f32, tag="tp", name="SP1")
                nc.tensor.transpose(SP1[:], X[:, 96:98], ident)
                CRED = loop.tile([2, 1], f32, tag="CRED")
                nc.vector.tensor_reduce(CRED[:], SP1[:], axis=AX.X, op=ALU.max)
                nc.vector.tensor_add(BT[nxt][:], BT[cur][:], CRED[:])
                TIN = loop.tile([2, K], f32, tag="TIN")
                nc.vector.tensor_scalar_add(TIN[:], ZERO2[:], BT[nxt][:])
                SP2 = tpool.tile([K, 2], f32, tag="tp", name="SP2")
                nc.tensor.transpose(SP2[:], TIN[:], ident[0:2, 0:2])
                nc.vector.tensor_copy(TB[nxt][:], SP2[:])

        with tc.For_i(0, T // DMAX, 1) as it:
            CCW = loop.tile([K, 2 * DMAX], f32, tag="CCW")
            nc.vector.tensor_copy(
                CCW[:], CC[:, bass.ds(2 * DMAX * it, 2 * DMAX)])
            c2w = loop.tile([BL, DMAX], f32, tag="c2w")
            nc.vector.tensor_copy(c2w[:], cum2[:, bass.ds(DMAX * it, DMAX)])
            for u in range(DMAX):
                step(CCW, c2w, u)

        dma(out_m[:, 0:BL], P[:])
        dma(out_m[0:BL, BL:BL + 1], OUTC[:])

    # Force Exp and Ln to resolve to the single table set that holds both,
    # so the scan never swaps ACT tables (1.3us per swap otherwise).
    import concourse.bacc as _bacc_mod
    _orig_tables = _bacc_mod.get_activation_tables

    def _patched_tables(arch):
        t = _orig_tables(arch)
        for name, funcs in t.items():
            if name != "natural_log_exp_and_others":
                funcs.discard(ACTF.Exp)
                funcs.discard(ACTF.Ln)
        return t

    _bacc_mod.get_activation_tables = _patched_tables
    try:
        nc.finalize()
    finally:
        _bacc_mod.get_activation_tables = _orig_tables
    return nc


def _get_program(ncores=NCORES):
    if ncores not in _CACHE:
        _CACHE[ncores] = build_program(ncores)
    return _CACHE[ncores]


def _derive_params(inputs):
    """Host-side float32 derivation of the small HSMM parameters."""
    g = lambda n: np.asarray(inputs[n], dtype=np.float32)
    ctx = g("context")

    def logsm(z):
        z = z - z.max(axis=-1, keepdims=True)
        return z - np.log(np.exp(z).sum(axis=-1, keepdims=True))

    zA = g("trans_logits") + 0.1 * np.tanh(
        (g("ctx_A_w") @ ctx + g("ctx_A_b")).reshape(K, K))
    A = np.exp(logsm(zA))
    zD = g("dur_logits") + 0.1 * np.tanh(
        (g("ctx_D_w") @ ctx + g("ctx_D_b")).reshape(K, DMAX))
    Dhat = np.exp(logsm(zD))
    mu_eff = g("mu") + 0.1 * (g("ctx_E_w") @ ctx + g("ctx_E_b")).reshape(K, NF)
    lv = g("log_var")
    var = np.maximum(lv, 0.0) + np.log1p(np.exp(-np.abs(lv))) + 1e-3
    inv = 1.0 / var
    M2 = -2.0 * mu_eff * inv
    bias_k = -0.5 * ((mu_eff * mu_eff * inv).sum(-1) + np.log(var).sum(-1)
                     + NF * LOG2PI)
    logpi = logsm(g("pi_logits"))

    pk = np.zeros((K, PW), dtype=np.float32)
    pk[:, O_A:O_A + K] = A
    pk[:, O_D:O_D + DMAX] = Dhat
    for c in range(2):
        pk[:, O_M1 + c * K:O_M1 + (c + 1) * K] = inv[:, c * K:(c + 1) * K].T
        pk[:, O_M2 + c * K:O_M2 + (c + 1) * K] = M2[:, c * K:(c + 1) * K].T
    pk[:, O_I:O_I + K] = np.eye(K, dtype=np.float32)
    pk[:, O_S] = bias_k
    pk[:, O_S + 1] = logpi
    pk[:, O_S + 2] = logpi.max()
    pk[:, O_S + 3] = logpi.max()
    return pk


def make_in_maps(inputs, ncores=NCORES):
    f8np = mybir.dt.np(f8)
    xq = np.asarray(inputs["x"], dtype=np.float32).astype(f8np)
    pk = _derive_params(inputs)
    maps = []
    for cix in range(ncores):
        pk_c = np.ascontiguousarray(pk[:, cix * CW:(cix + 1) * CW])
        maps.append({
            "blob": np.concatenate([
                xq[cix * BL:(cix + 1) * BL].reshape(-1),
                pk_c.reshape(-1).view(f8np)]),
        })
    return maps


def assemble_output(results):
    out = np.empty(B, np.float32)
    for cix, r in enumerate(results):
        m = np.asarray(r["out_m"], np.float32)      # [K, BL+1]
        for s in range(BL):
            out[cix * BL + s] = m[s, BL] + np.float32(
                np.log(m[:, s].sum(dtype=np.float32)))
    return out


def kernel(**inputs):
    from concourse.bass_utils import run_bass_kernel_spmd
    nc = _get_program(NCORES)
    in_maps = make_in_maps(inputs, NCORES)
    res = run_bass_kernel_spmd(nc, in_maps, list(range(NCORES)))
    return assemble_output(res.results)
